# revision 1
# baseline (speedup 1.0000x reference)
"""MinibatchDiscrimination kernel for 8 Trainium2 NeuronCores.

reference:
    m = einsum('bi,iok->bok', x, T)          # B=128, IN=1024, OUT=512, K=16
    norm[i,j,o] = sum_k |m[j,o,k] - m[i,o,k]|
    o_b = sum_i exp(-norm) - 1               # [B, OUT]
    out = concat([x, o_b], axis=1)           # [128, 1536]

Sharding: each core owns OUT/8 = 64 output features (zero communication).

Per-core pipeline (pair-matmul, strictly-upper-triangular):
  1. GEMM on PE: m[b, f] = x @ T_c, f = o_local*16 + k (F = 1024, 8 f-tiles).
  2. Pair differences on PE: for f-tile t, diff[f, pair] = m_t.T @ psel where
     psel[b, (i,j)] = +1{b==i} - 1{b==j} over the 8128 pairs i<j. Streamed in
     [128, 512] PSUM chunks.
  3. |diff| -> SBUF bf16: ACT tiles use one Abs op per chunk; DVE tiles use
     two fused ops (relu(d), relu(-d)) into separate planes (the add is
     folded into the k-reduce contraction width).
  4. k-reduce + i-stacking on PE: per i one matmul over its pair block,
     selector S32_a [128, 32] with tile_position=(0, 32q) packs 16 i's into
     one [128, 128] group (row = 32*(isub//4) + 8*(isub%4) + osub); four
     groups share one PSUM bank [128, 512]; matmul start=True zeroes the
     bank once, so unwritten (j <= i) columns are exact zeros.
  5. exp(-norm) on ACT over [128, 512]; zeros exp to exactly 1.0 -> the
     deterministic junk is removed host-side (po[o,j] -= 128-j, rowsum -= i+1).
  6. Column sums: selector matmul S2_t [128, 64] accumulates over everything
     into PSUM [64, 128]. Row sums: DVE tensor_reduce -> [128, 64] table.
  7. Host: o_b[j, o] = (po[o, j] - (128-j)) + reindexed rowsums.
i==j pairs are never computed, so no "-1" correction is needed.
"""

import numpy as np
import ml_dtypes

import concourse.bass as bass
import concourse.tile as tile
from concourse import mybir
from concourse.bass_utils import run_bass_kernel_spmd

BF16 = mybir.dt.bfloat16
F32 = mybir.dt.float32
A = mybir.AluOpType
AF = mybir.ActivationFunctionType

B = 128
IN = 1024
OUT = 512
K = 16
NCORES = 8
OC = OUT // NCORES       # 64
F = OC * K               # 1024
NT = F // 128            # 8 f-tiles
NCI = IN // 128          # 8 contraction chunks
NPAIR = (B * (B - 1)) // 2   # 8128 strictly-upper pairs
CHUNK = 512
NCHUNK = (NPAIR + CHUNK - 1) // CHUNK   # 16 (last = 448)

# which f-tiles run their |diff| on DVE (two relu planes) vs ACT (one Abs op)
DVE_TILES = (False, False, False, False, False, False, False, False)
SUPER = 1024                                  # abs op width (2 PSUM banks)
NSUPER = (NPAIR + SUPER - 1) // SUPER         # 8 (last = 960)


def _pair_base(i):
    return i * 127 - (i * (i - 1)) // 2


def _split_excess_waits(nc, max_waits=1):
    """This walrus build rejects instructions carrying more than one sem
    wait; hoist extras onto preceding NoOps on the same engine."""
    for fn in nc.m.functions:
        for blk in fn.blocks:
            new_insts = []
            for inst in blk.instructions:
                si = inst.sync_info
                if si and si.on_wait and len(si.on_wait) > max_waits:
                    waits = list(si.on_wait)
                    extra, keep = waits[:-max_waits], waits[-max_waits:]
                    k = 0
                    while extra:
                        chunk, extra = extra[:max_waits], extra[max_waits:]
                        nop = mybir.InstNoOp(
                            name=f"{inst.name}-ws{k}", engine=inst.engine,
                            ins=[], outs=[],
                            sync_info=mybir.SyncInfo(on_wait=chunk, on_update=[]))
                        nc.register_instruction(nop)
                        new_insts.append(nop)
                        k += 1
                    inst.sync_info = mybir.SyncInfo(
                        on_wait=keep, on_update=list(si.on_update))
                new_insts.append(inst)
            blk.instructions[:] = new_insts


def _make_pd_abs_steps(nc, pools, t, m_bf, psel_sb):
    """Returns (absd_tile, steps): each step emits one pair-diff chunk
    matmul + its |.| op(s) when called."""
    work, ework, pdiff, pnorm = pools
    dve = DVE_TILES[t]
    planes = 2 if dve else 1
    absd = work.tile([128, planes, NPAIR], BF16, tag="absd")

    def step(c):
        lo = c * SUPER
        w = min(SUPER, NPAIR - lo)
        pd = pdiff.tile([128, SUPER], F32, tag="pd")
        # one matmul per PSUM bank (N <= 512), then one wide |.| op over
        # both banks to amortize the ACT/DVE per-op bubble
        for h in range(0, w, CHUNK):
            hw = min(CHUNK, w - h)
            nc.tensor.matmul(pd[:, h:h + hw], m_bf[:, 128 * t:128 * (t + 1)],
                             psel_sb[:, lo + h:lo + h + hw],
                             start=True, stop=True)
        if dve:
            nc.vector.tensor_scalar(absd[:, 0, lo:lo + w], pd[:, 0:w],
                                    0.0, None, op0=A.max)
            nc.vector.tensor_scalar(absd[:, 1, lo:lo + w], pd[:, 0:w],
                                    -1.0, 0.0, op0=A.mult, op1=A.max)
        else:
            nc.scalar.activation(absd[:, 0, lo:lo + w], pd[:, 0:w], AF.Abs)

    return absd, [lambda c=c: step(c) for c in range(NSUPER)]


def _emit_kred(nc, pools, t, absd, s32_sb, s2_sb, po, rs_all, weave=None):
    """k-reduce (packed, strip-interleaved) + exp + row/col sums for tile t.
    `weave` is a list of pending pair-diff steps for the NEXT tile; they are
    interleaved into the PE stream so the next tile's abs pass (ACT/DVE)
    overlaps this tile's k-reduce (PE)."""
    work, ework, pdiff, pnorm = pools
    dve = DVE_TILES[t]
    weave = list(weave or [])
    n_mm = 8 * 16 * (2 if dve else 1)
    stride = max(1, n_mm // (len(weave) + 1)) if weave else 0
    mm_count = 0

    def tick():
        nonlocal mm_count
        mm_count += 1
        if weave and stride and mm_count % stride == 0:
            weave.pop(0)()
    for G in range(2):
        pn = pnorm.tile([128, 512], F32, tag="pn")
        # zero the full tile: cells no matmul writes (j <= i) must read as
        # exact 0 so exp gives exactly 1.0 (host subtracts the known count)
        nc.vector.memset(pn[:], 0.0)
        first = True
        for gl in range(4):
            ig = 4 * G + gl
            for idx in range(16):
                # strip-interleaved: consecutive matmuls hit different
                # 32-col PE strips (q fastest) so they run concurrently
                q, a = idx % 4, idx // 4
                i = 16 * ig + 4 * a + q
                if i >= B - 1:
                    continue
                w = 127 - i
                bs = _pair_base(i)
                out_ap = pn[32 * q:32 * q + 32,
                            128 * gl + i + 1:128 * (gl + 1)]
                last = (gl == 3 and idx == 15)
                nc.tensor.matmul(
                    out_ap, s32_sb[a][:], absd[:, 0, bs:bs + w],
                    start=first, stop=(last and not dve),
                    tile_position=(0, 32 * q), skip_group_check=True)
                first = False
                tick()
                if dve:
                    # second relu plane accumulates into the same columns
                    nc.tensor.matmul(
                        out_ap, s32_sb[a][:], absd[:, 1, bs:bs + w],
                        start=False, stop=last,
                        tile_position=(0, 32 * q), skip_group_check=True)
                    tick()
        e = ework.tile([128, 512], BF16, tag="e")
        nc.scalar.activation(e[:], pn[:], AF.Exp, scale=-1.0)
        # row sums over j within each igroup -> rs_all[:, 8*ig + t]
        rs_view = rs_all.rearrange("p (ig tt) -> p ig tt", tt=8)
        nc.vector.tensor_reduce(
            rs_view[:, 4 * G:4 * G + 4, t],
            e[:].rearrange("p (g j) -> p g j", g=4), op=A.add,
            axis=mybir.AxisListType.X)
        for gl in range(4):
            ig = 4 * G + gl
            nc.tensor.matmul(po[:], s2_sb[t][:],
                             e[:, 128 * gl:128 * (gl + 1)],
                             start=(t == 0 and ig == 0),
                             stop=(t == NT - 1 and ig == 7))
    # flush any unwoven pair-diff steps for the next tile
    for stp in weave:
        stp()


def _build_program():
    nc = bass.Bass()
    xT_d = nc.dram_tensor("xt", [IN, B], BF16, kind="ExternalInput")
    tc_d = nc.dram_tensor("tc", [IN, F], BF16, kind="ExternalInput")
    psel_d = nc.dram_tensor("psel", [B, NPAIR], BF16, kind="ExternalInput")
    s32_d = nc.dram_tensor("s32", [4, 128, 32], BF16, kind="ExternalInput")
    s2_d = nc.dram_tensor("s2", [NT, 128, OC], BF16, kind="ExternalInput")
    po_d = nc.dram_tensor("po", [OC, B], F32, kind="ExternalOutput")
    rs_d = nc.dram_tensor("rs", [128, 64], F32, kind="ExternalOutput")

    with tile.TileContext(nc) as tc:
        with (
            tc.tile_pool(name="cst", bufs=1) as cst,
            tc.tile_pool(name="work", bufs=3) as work,
            tc.tile_pool(name="ework", bufs=4) as ework,
            tc.tile_pool(name="pgemm", bufs=1, space="PSUM") as pgemm,
            tc.tile_pool(name="pdiff", bufs=2, space="PSUM") as pdiff,
            tc.tile_pool(name="pnorm", bufs=2, space="PSUM") as pnorm,
            tc.tile_pool(name="pob", bufs=1, space="PSUM") as pob,
        ):
            xT_sb, tc_sb = [], []
            for ci in range(NCI):
                t_ = cst.tile([128, F], BF16, tag=f"tc{ci}")
                nc.sync.dma_start(t_[:], tc_d[128 * ci:128 * (ci + 1), :])
                tc_sb.append(t_)
                x_ = cst.tile([128, B], BF16, tag=f"xt{ci}")
                nc.sync.dma_start(x_[:], xT_d[128 * ci:128 * (ci + 1), :])
                xT_sb.append(x_)
            # per-chunk DMA so the first pair-diff matmul can start as soon
            # as its slice (and m_bf) lands, not after the full 2MB
            psel_sb = cst.tile([128, NPAIR], BF16, tag="psel")
            for cch in range(NCHUNK):
                lo = cch * CHUNK
                w = min(CHUNK, NPAIR - lo)
                nc.sync.dma_start(psel_sb[:, lo:lo + w], psel_d[:, lo:lo + w])
            s32_sb = []
            for a in range(4):
                t_ = cst.tile([128, 32], BF16, tag=f"s32_{a}")
                nc.sync.dma_start(t_[:], s32_d[a])
                s32_sb.append(t_)
            s2_sb = []
            for t in range(NT):
                t_ = cst.tile([128, OC], BF16, tag=f"s2{t}")
                nc.sync.dma_start(t_[:], s2_d[t])
                s2_sb.append(t_)

            # ---- GEMM: m[b, f] = x @ T_c ----
            m_bf = cst.tile([128, F], BF16, tag="mbf")
            for half in range(2):
                ps = pgemm.tile([128, 512], F32, tag="pg")
                for ci in range(NCI):
                    nc.tensor.matmul(
                        ps[:], xT_sb[ci][:],
                        tc_sb[ci][:, 512 * half:512 * (half + 1)],
                        start=(ci == 0), stop=(ci == NCI - 1))
                nc.scalar.activation(m_bf[:, 512 * half:512 * (half + 1)],
                                     ps[:], AF.Copy, scale=1.0)

            po = pob.tile([OC, B], F32, tag="po")
            rs_all = cst.tile([128, 64], F32, tag="rs")

            # software pipeline: tile t's k-reduce (PE) interleaves the
            # pair-diff chunks of tile t+1, so t+1's abs pass (ACT/DVE)
            # overlaps t's k-reduce instead of serializing after it
            pools = (work, ework, pdiff, pnorm)
            absd0, steps0 = _make_pd_abs_steps(nc, pools, 0, m_bf, psel_sb)
            for s in steps0:
                s()
            cur_absd = absd0
            for t in range(NT):
                if t + 1 < NT:
                    nxt_absd, nxt_steps = _make_pd_abs_steps(
                        nc, pools, t + 1, m_bf, psel_sb)
                else:
                    nxt_absd, nxt_steps = None, []
                _emit_kred(nc, pools, t, cur_absd, s32_sb, s2_sb,
                           po, rs_all, weave=nxt_steps)
                cur_absd = nxt_absd

            po_sb = cst.tile([OC, B], F32, tag="posb")
            nc.vector.tensor_copy(po_sb[:], po[:])
            nc.sync.dma_start(po_d[:], po_sb[:])
            nc.sync.dma_start(rs_d[:], rs_all[:])

    _split_excess_waits(nc)
    return nc


def _host_consts():
    psel = np.zeros((B, NPAIR), np.float32)
    col = 0
    for i in range(B - 1):
        w = 127 - i
        psel[i, col:col + w] = 1.0
        psel[np.arange(i + 1, 128), np.arange(col, col + w)] = -1.0
        col += w
    s32 = np.zeros((4, 128, 32), np.float32)
    for a in range(4):
        for osub in range(8):
            s32[a, 16 * osub:16 * (osub + 1), 8 * a + osub] = 1.0
    s2 = np.zeros((NT, 128, OC), np.float32)
    for t in range(NT):
        for p in range(128):
            s2[t, p, 8 * t + (p % 8)] = 1.0
    return (psel.astype(ml_dtypes.bfloat16), s32.astype(ml_dtypes.bfloat16),
            s2.astype(ml_dtypes.bfloat16))


_CACHE = {}


def _get_cached():
    if "nc" not in _CACHE:
        _CACHE["nc"] = _build_program()
        _CACHE["consts"] = _host_consts()
        # rowsum reindex: rs_all[p, 8*ig + t] belongs to
        # i = 16*ig + 4*a + q with p = 32*q + 8*a + osub, o = 8*t + osub
        p_idx = np.arange(128)
        q, rem = p_idx // 32, p_idx % 32
        a_, osub = rem // 8, rem % 8
        cols = np.arange(64)
        ig, t_ = cols // 8, cols % 8
        i_map = 16 * ig[None, :] + 4 * a_[:, None] + q[:, None]   # [128, 64]
        o_map = 8 * t_[None, :] + osub[:, None]                   # [128, 64]
        _CACHE["i_map"] = i_map
        _CACHE["o_map"] = o_map
    return _CACHE


def kernel(x: np.ndarray, T: np.ndarray, _trace=False, _tmpdir=None) -> np.ndarray:
    x = np.asarray(x, dtype=np.float32)
    T = np.asarray(T, dtype=np.float32)
    c = _get_cached()
    nc = c["nc"]
    psel, s32, s2 = c["consts"]

    xt = np.ascontiguousarray(x.T).astype(ml_dtypes.bfloat16)
    in_maps = []
    for cr in range(NCORES):
        tc_c = np.ascontiguousarray(
            T[:, OC * cr:OC * (cr + 1), :].reshape(IN, F)
        ).astype(ml_dtypes.bfloat16)
        in_maps.append({"xt": xt, "tc": tc_c, "psel": psel,
                        "s32": s32, "s2": s2})

    kw = {}
    if _trace:
        kw = dict(trace=True, tmpdir=_tmpdir)
    res = run_bass_kernel_spmd(nc, in_maps, list(range(NCORES)), **kw)

    jj = np.arange(B, dtype=np.float32)
    junk_col = (B - jj)[None, :]          # po[o, j] junk = 128 - j
    i_map, o_map = c["i_map"], c["o_map"]
    o_b = np.empty((B, OUT), np.float32)
    for cr in range(NCORES):
        r = res.results[cr]
        po = r["po"] - junk_col                       # [64, 128] colsums
        ob_c = po.T.copy()                            # [j, o_local]
        rows = r["rs"] - (i_map + 1)                  # rowsums minus junk
        np.add.at(ob_c, (i_map.ravel(), o_map.ravel()), rows.ravel())
        o_b[:, OC * cr:OC * (cr + 1)] = ob_c
    out = np.concatenate([x, o_b], axis=1)
    if _trace:
        return out, res
    return out



# revision 7
# speedup vs baseline: 1.0483x; 1.0483x over previous
"""MinibatchDiscrimination kernel for 8 Trainium2 NeuronCores.

reference:
    m = einsum('bi,iok->bok', x, T)          # B=128, IN=1024, OUT=512, K=16
    norm[i,j,o] = sum_k |m[j,o,k] - m[i,o,k]|
    o_b = sum_i exp(-norm) - 1               # [B, OUT]
    out = concat([x, o_b], axis=1)           # [128, 1536]

Sharding: each core owns OUT/8 = 64 output features (zero communication).

Per-core pipeline, pairs-on-partitions layout (v2):
  f = k*64 + o_local (k-major) so the k-reduction folds keep a contiguous
  64-wide o innermost dim (DVE 2x_1p eligible).
  1. GEMM on PE (fp8): m8[b, f] = x8 @ T8, cast PSUM -> fp8 SBUF.
  2. Pair-diff on PE (fp8): for each 128-pair chunk c, stationary
     psel_c [128b, 128pair] (+1 at i, -1 at j), moving m8 [128b, 1024f]:
     pd_c [128pair, 1024f] fp32 PSUM (2 matmuls of 512).
  3. |pd| -> bf16 SBUF, one op per chunk, engine chosen per-chunk from a
     static pattern balancing ACT (activation Abs) / DVE (tensor_scalar
     abs_max) / Pool (gpsimd tensor_scalar abs_max).
  4. k-reduce: per 16-chunk group, 4 halving tensor_tensor adds on DVE
     (bf16, packed innermost 64 -> 2x_1p) -> norm [128, 16c*64o].
  5. exp(-norm) on ACT -> e fp8 (one op per group).
  6. j-sum on PE: per chunk, matmul(po2 [64o, 128b] += e_c^T @ zt_c) where
     zt_c [128pair, 128b] = 1 at both i and j of the pair. Both pair
     orientations in one matmul; pad pairs have zero zt rows.
  7. Host: o_b[j, o] = po2[o, j]; out = concat(x, o_b). No corrections:
     self-pairs never computed, pads contribute 0.
"""

import numpy as np
import ml_dtypes

import concourse.bass as bass
import concourse.tile as tile
from concourse import mybir
from concourse.bass_utils import run_bass_kernel_spmd

BF16 = mybir.dt.bfloat16
F32 = mybir.dt.float32
F8 = mybir.dt.float8e4
NP_F8 = ml_dtypes.float8_e4m3
A = mybir.AluOpType
AF = mybir.ActivationFunctionType

B = 128
IN = 1024
OUT = 512
K = 16
NCORES = 8
OC = OUT // NCORES       # 64
F = OC * K               # 1024 (f = k*64 + o)
NCI = IN // 128          # 8 contraction chunks
NPAIR = (B * (B - 1)) // 2   # 8128 strictly-upper pairs
NCHUNK = 64              # 128-pair chunks (last 64 pairs are zero-pad)
GROUP = 16               # chunks per fold group
NGROUP = NCHUNK // GROUP

# Within each 16-chunk group: the first N_ACT chunks go ACT-abs -> AB ->
# DVE fold tree; the rest are single fused DVE tensor_reduce(|.|, sum k)
# straight from PSUM to the norm tile. (gpsimd cannot read PSUM at all.)
N_ACT = 11


def _split_excess_waits(nc, max_waits=1):
    """This walrus build rejects instructions carrying more than one sem
    wait; hoist extras onto preceding NoOps on the same engine."""
    for fn in nc.m.functions:
        for blk in fn.blocks:
            new_insts = []
            for inst in blk.instructions:
                si = inst.sync_info
                if si and si.on_wait and len(si.on_wait) > max_waits:
                    waits = list(si.on_wait)
                    extra, keep = waits[:-max_waits], waits[-max_waits:]
                    k = 0
                    while extra:
                        chunk, extra = extra[:max_waits], extra[max_waits:]
                        nop = mybir.InstNoOp(
                            name=f"{inst.name}-ws{k}", engine=inst.engine,
                            ins=[], outs=[],
                            sync_info=mybir.SyncInfo(on_wait=chunk, on_update=[]))
                        nc.register_instruction(nop)
                        new_insts.append(nop)
                        k += 1
                    inst.sync_info = mybir.SyncInfo(
                        on_wait=keep, on_update=list(si.on_update))
                new_insts.append(inst)
            blk.instructions[:] = new_insts


def _build_program():
    nc = bass.Bass()
    xT_d = nc.dram_tensor("xt", [IN, B], F8, kind="ExternalInput")
    tc_d = nc.dram_tensor("tc", [IN, F], F8, kind="ExternalInput")
    psel_d = nc.dram_tensor("psel", [B, NCHUNK * 128], F8, kind="ExternalInput")
    zt_d = nc.dram_tensor("zt", [128, NCHUNK * B], F8, kind="ExternalInput")
    po_d = nc.dram_tensor("po", [OC, B], F32, kind="ExternalOutput")

    with tile.TileContext(nc) as tc:
        with (
            tc.tile_pool(name="cst", bufs=1) as cst,
            tc.tile_pool(name="ab", bufs=2) as abp,
            tc.tile_pool(name="fold", bufs=2) as fold,
            tc.tile_pool(name="ep", bufs=2) as ep,
            tc.tile_pool(name="pgemm", bufs=2, space="PSUM") as pgemm,
            tc.tile_pool(name="pdiff", bufs=2, space="PSUM") as pdiff,
            tc.tile_pool(name="pob", bufs=1, space="PSUM") as pob,
        ):
            xT_sb, tc_sb = [], []
            for ci in range(NCI):
                x_ = cst.tile([128, B], F8, tag=f"xt{ci}")
                nc.sync.dma_start(x_[:], xT_d[128 * ci:128 * (ci + 1), :])
                xT_sb.append(x_)
                t_ = cst.tile([128, F], F8, tag=f"tc{ci}")
                nc.sync.dma_start(t_[:], tc_d[128 * ci:128 * (ci + 1), :])
                tc_sb.append(t_)
            # per-chunk-pair DMA so early pair-diff matmuls start as soon as
            # their slice lands
            psel_sb = cst.tile([128, NCHUNK * 128], F8, tag="psel")
            for c2 in range(NCHUNK // 2):
                lo = c2 * 256
                nc.sync.dma_start(psel_sb[:, lo:lo + 256],
                                  psel_d[:, lo:lo + 256])
            zt_sb = cst.tile([128, NCHUNK * B], F8, tag="zt")
            for g in range(NGROUP):
                lo = g * GROUP * B
                nc.sync.dma_start(zt_sb[:, lo:lo + GROUP * B],
                                  zt_d[:, lo:lo + GROUP * B])

            # ---- GEMM: m8[b, f] = x8 @ T8 ----
            m8 = cst.tile([128, F], F8, tag="m8")
            for half in range(2):
                ps = pgemm.tile([128, 512], F32, tag="pg")
                for ci in range(NCI):
                    nc.tensor.matmul(
                        ps[:], xT_sb[ci][:],
                        tc_sb[ci][:, 512 * half:512 * (half + 1)],
                        start=(ci == 0), stop=(ci == NCI - 1))
                nc.scalar.activation(m8[:, 512 * half:512 * (half + 1)],
                                     ps[:], AF.Copy, scale=1.0)

            po = pob.tile([OC, B], F32, tag="po")

            def emit_group_pd_abs(g):
                """pair-diff for the 16 chunks of group g. First N_ACT
                chunks: ACT abs into the AB tile (folded later on DVE); the
                rest: one fused DVE tensor_reduce(|.|, sum k) straight into
                the group's norm tile."""
                ab = abp.tile([128, N_ACT * F], BF16, tag="ab")
                nrm = fold.tile([128, GROUP * OC], BF16, tag="nrm")
                for cc in range(GROUP):
                    c = GROUP * g + cc
                    pd = pdiff.tile([128, F], F32, tag="pd")
                    for h in range(2):
                        nc.tensor.matmul(
                            pd[:, 512 * h:512 * (h + 1)],
                            psel_sb[:, 128 * c:128 * (c + 1)],
                            m8[:, 512 * h:512 * (h + 1)],
                            start=True, stop=True)
                    if cc < N_ACT:
                        nc.scalar.activation(ab[:, F * cc:F * (cc + 1)],
                                             pd[:], AF.Abs)
                    else:
                        with nc.allow_low_precision(reason="norm in bf16"):
                            nc.vector.tensor_reduce(
                                nrm[:, OC * cc:OC * (cc + 1)],
                                pd[:].rearrange("p (k o) -> p o k", k=K),
                                op=A.add, axis=mybir.AxisListType.X,
                                apply_absolute_value=True)
                return ab, nrm

            def emit_group_reduce(g, ab, nrm):
                """fold k (16 -> 1) on DVE for the ACT chunks, exp on ACT,
                j-sum matmuls on PE."""
                v = ab[:].rearrange("p (c k o) -> p c k o", c=N_ACT, k=K)
                n1 = fold.tile([128, N_ACT * 8 * OC], BF16, tag="n1")
                v1 = n1[:].rearrange("p (c k o) -> p c k o", c=N_ACT, k=8)
                n2 = fold.tile([128, N_ACT * 4 * OC], BF16, tag="n2")
                v2 = n2[:].rearrange("p (c k o) -> p c k o", c=N_ACT, k=4)
                n3 = fold.tile([128, N_ACT * 2 * OC], BF16, tag="n3")
                v3 = n3[:].rearrange("p (c k o) -> p c k o", c=N_ACT, k=2)
                vn = nrm[:, 0:N_ACT * OC].rearrange(
                    "p (c k o) -> p c k o", c=N_ACT, k=1)
                with nc.allow_low_precision(reason="norm folds in bf16"):
                    nc.vector.tensor_tensor(v1, v[:, :, 0:8], v[:, :, 8:16],
                                            op=A.add)
                    nc.vector.tensor_tensor(v2, v1[:, :, 0:4], v1[:, :, 4:8],
                                            op=A.add)
                    nc.vector.tensor_tensor(v3, v2[:, :, 0:2], v2[:, :, 2:4],
                                            op=A.add)
                    nc.vector.tensor_tensor(vn, v3[:, :, 0:1], v3[:, :, 1:2],
                                            op=A.add)
                e = ep.tile([128, GROUP * OC], F8, tag="e")
                nc.scalar.activation(e[:], nrm[:], AF.Exp, scale=-1.0)
                for cc in range(GROUP):
                    c = GROUP * g + cc
                    nc.tensor.matmul(
                        po[:], e[:, OC * cc:OC * (cc + 1)],
                        zt_sb[:, B * c:B * (c + 1)],
                        start=(c == 0), stop=(c == NCHUNK - 1))

            # software pipeline: PE runs group g+1's pair-diff while
            # ACT/DVE/Pool chew group g
            prev = emit_group_pd_abs(0)
            for g in range(NGROUP):
                nxt = emit_group_pd_abs(g + 1) if g + 1 < NGROUP else None
                emit_group_reduce(g, *prev)
                prev = nxt

            po_sb = cst.tile([OC, B], F32, tag="posb")
            nc.vector.tensor_copy(po_sb[:], po[:])
            nc.sync.dma_start(po_d[:], po_sb[:])

    _split_excess_waits(nc)
    return nc


def _host_consts():
    # pair enumeration: strict upper, i-major: p = base(i) + (j - i - 1)
    ii, jj = np.triu_indices(B, k=1)          # [NPAIR] each, i-major order
    psel = np.zeros((B, NCHUNK * 128), np.float32)
    p = np.arange(NPAIR)
    psel[ii, p] = 1.0
    psel[jj, p] = -1.0
    zt = np.zeros((128, NCHUNK * B), np.float32)
    c = p // 128
    r = p % 128
    zt[r, c * B + ii] = 1.0
    zt[r, c * B + jj] = 1.0
    return psel.astype(NP_F8), zt.astype(NP_F8)


_CACHE = {}


def _get_cached():
    if "nc" not in _CACHE:
        _CACHE["nc"] = _build_program()
        _CACHE["consts"] = _host_consts()
    return _CACHE


def kernel(x: np.ndarray, T: np.ndarray, _trace=False, _tmpdir=None) -> np.ndarray:
    x = np.asarray(x, dtype=np.float32)
    T = np.asarray(T, dtype=np.float32)
    c = _get_cached()
    nc = c["nc"]
    psel, zt = c["consts"]

    xt8 = np.ascontiguousarray(x.T).astype(NP_F8)
    in_maps = []
    for cr in range(NCORES):
        # f = k*64 + o_local: transpose the (o, k) axes before flattening
        tc8 = np.ascontiguousarray(
            T[:, OC * cr:OC * (cr + 1), :].transpose(0, 2, 1).reshape(IN, F)
        ).astype(NP_F8)
        in_maps.append({"xt": xt8, "tc": tc8, "psel": psel, "zt": zt})

    kw = {}
    if _trace:
        kw = dict(trace=True, tmpdir=_tmpdir)
    res = run_bass_kernel_spmd(nc, in_maps, list(range(NCORES)), **kw)

    o_b = np.empty((B, OUT), np.float32)
    for cr in range(NCORES):
        o_b[:, OC * cr:OC * (cr + 1)] = res.results[cr]["po"].T
    out = np.concatenate([x, o_b], axis=1)
    if _trace:
        return out, res
    return out


# revision 9
# speedup vs baseline: 1.1163x; 1.0649x over previous
"""MinibatchDiscrimination kernel for 8 Trainium2 NeuronCores (v3 hybrid).

reference:
    m = einsum('bi,iok->bok', x, T)          # B=128, IN=1024, OUT=512, K=16
    norm[i,j,o] = sum_k |m[j,o,k] - m[i,o,k]|
    o_b = sum_i exp(-norm) - 1               # [B, OUT]
    out = concat([x, o_b], axis=1)           # [128, 1536]

Sharding: each core owns OUT/8 = 64 output features (zero communication).
f = k*64 + o_local (k-major).

Hybrid split of the 8128 strict-upper pairs:
- PE-share: pairs with i < I0 (p < PB) in f-layout. Pair-diff per f-tile
  t: absd_t [128f, PB] (ACT abs). k-reduce ON THE PE: per (i, t) one
  matmul with the fixed selector s64[p, o]=1{p%64==o}, accumulating all 8
  tiles into packed PSUM pn[big] [128=2i*64o, 512=4sub*128j]. exp on ACT,
  column sums via s64 again into po, row sums via DVE tensor_reduce.
  Host subtracts deterministic junk (memset cells j<=i read exp(0)=1).
- pairs-share: remaining pairs in pairs-on-partitions chunks of 128.
  Per chunk: pair-diff matmul -> pd [128pair, 1024f] PSUM; first N_ACT
  chunks of each group: ACT abs -> DVE fold tree; rest: one fused DVE
  tensor_reduce(|.|, sum k). exp(-norm) on ACT -> e fp8; j-sum matmul
  po += e_c^T @ zt_c (zt has 1s at both i and j; pad pairs are zero).
Host: o_b[j, o] = po[o, j] - junk + rowsum scatter; out = concat(x, o_b).
"""

import numpy as np
import ml_dtypes

import concourse.bass as bass
import concourse.tile as tile
from concourse import mybir
from concourse.bass_utils import run_bass_kernel_spmd

BF16 = mybir.dt.bfloat16
F32 = mybir.dt.float32
F8 = mybir.dt.float8e4
NP_F8 = ml_dtypes.float8_e4m3
A = mybir.AluOpType
AF = mybir.ActivationFunctionType

B = 128
IN = 1024
OUT = 512
K = 16
NCORES = 8
OC = OUT // NCORES       # 64
F = OC * K               # 1024 (f = k*64 + o)
NCI = IN // 128          # 8
NPAIR = (B * (B - 1)) // 2   # 8128

I0 = 16                  # PE-share: pairs with i < I0
PB = I0 * 127 - (I0 * (I0 - 1)) // 2   # 1912 pairs
NBIG = (I0 + 7) // 8     # pn tiles (8 i's each)

NP_PAIRS = NPAIR - PB    # 6216 pairs-share pairs
NCHUNK = (NP_PAIRS + 127) // 128       # 49 chunks
PSEL_COLS = PB + NCHUNK * 128          # 8184
GROUPS = (13, 12, 12, 12)              # chunks per fold group
N_ACT = 7                              # ACT-abs chunks per group


def _split_excess_waits(nc, max_waits=1):
    """This walrus build rejects instructions carrying more than one sem
    wait; hoist extras onto preceding NoOps on the same engine."""
    for fn in nc.m.functions:
        for blk in fn.blocks:
            new_insts = []
            for inst in blk.instructions:
                si = inst.sync_info
                if si and si.on_wait and len(si.on_wait) > max_waits:
                    waits = list(si.on_wait)
                    extra, keep = waits[:-max_waits], waits[-max_waits:]
                    k = 0
                    while extra:
                        chunk, extra = extra[:max_waits], extra[max_waits:]
                        nop = mybir.InstNoOp(
                            name=f"{inst.name}-ws{k}", engine=inst.engine,
                            ins=[], outs=[],
                            sync_info=mybir.SyncInfo(on_wait=chunk, on_update=[]))
                        nc.register_instruction(nop)
                        new_insts.append(nop)
                        k += 1
                    inst.sync_info = mybir.SyncInfo(
                        on_wait=keep, on_update=list(si.on_update))
                new_insts.append(inst)
            blk.instructions[:] = new_insts


def _build_program():
    nc = bass.Bass()
    xT_d = nc.dram_tensor("xt", [IN, B], F8, kind="ExternalInput")
    tc_d = nc.dram_tensor("tc", [IN, F], F8, kind="ExternalInput")
    psel_d = nc.dram_tensor("psel", [B, PSEL_COLS], F8, kind="ExternalInput")
    zt_d = nc.dram_tensor("zt", [128, NCHUNK * B], F8, kind="ExternalInput")
    s64_d = nc.dram_tensor("s64", [128, OC], BF16, kind="ExternalInput")
    po_d = nc.dram_tensor("po", [OC, B], F32, kind="ExternalOutput")
    rs_d = nc.dram_tensor("rs", [128, 4 * NBIG], F32, kind="ExternalOutput")

    with tile.TileContext(nc) as tc:
        with (
            tc.tile_pool(name="cst", bufs=1) as cst,
            tc.tile_pool(name="ab", bufs=2) as abp,
            tc.tile_pool(name="fold", bufs=2) as fold,
            tc.tile_pool(name="ep", bufs=2) as ep,
            tc.tile_pool(name="pdiff", bufs=2, space="PSUM") as pdiff,
            tc.tile_pool(name="pn", bufs=1, space="PSUM") as pnp,
            tc.tile_pool(name="pob", bufs=1, space="PSUM") as pob,
        ):
            # Few large DMAs: the ~600ns/descriptor fixed latency dominates
            # small transfers. xt/tc land as single tiles via 3D APs.
            s64_sb = cst.tile([128, OC], BF16, tag="s64")
            nc.sync.dma_start(s64_sb[:], s64_d[:, :])
            xt_sb = cst.tile([128, NCI * B], F8, tag="xt")
            nc.sync.dma_start(
                xt_sb[:].rearrange("p (c b) -> p c b", c=NCI),
                xT_d[:].rearrange("(c p) b -> p c b", p=128))
            tc_sb = cst.tile([128, NCI * F], F8, tag="tc")
            tcv = tc_sb[:].rearrange("p (c f) -> p c f", c=NCI)
            tdv = tc_d[:].rearrange("(c p) f -> p c f", p=128)
            nc.sync.dma_start(tcv[:, 0:4], tdv[:, 0:4])
            nc.sync.dma_start(tcv[:, 4:8], tdv[:, 4:8])
            psel_sb = cst.tile([128, PSEL_COLS], F8, tag="psel")
            for lo in range(0, PSEL_COLS, 2048):
                w = min(2048, PSEL_COLS - lo)
                nc.sync.dma_start(psel_sb[:, lo:lo + w],
                                  psel_d[:, lo:lo + w])
            zt_sb = cst.tile([128, NCHUNK * B], F8, tag="zt")
            for lo in range(0, NCHUNK * B, 2048):
                w = min(2048, NCHUNK * B - lo)
                nc.sync.dma_start(zt_sb[:, lo:lo + w], zt_d[:, lo:lo + w])

            # ---- GEMM: m8[b, f] = x8 @ T8 (psum tile from the pd pool) ----
            m8 = cst.tile([128, F], F8, tag="m8")
            ps = pdiff.tile([128, F], F32, tag="pd")
            for ci in range(NCI):
                for half in range(2):
                    nc.tensor.matmul(
                        ps[:, 512 * half:512 * (half + 1)],
                        xt_sb[:, B * ci:B * (ci + 1)],
                        tc_sb[:, F * ci + 512 * half:F * ci + 512 * (half + 1)],
                        start=(ci == 0), stop=(ci == NCI - 1),
                        skip_group_check=True)
            nc.scalar.activation(m8[:], ps[:], AF.Copy, scale=1.0)

            po = pob.tile([OC, B], F32, tag="po")
            pn0 = pnp.tile([128, 512], F32, tag="pn0")
            pn1 = pnp.tile([128, 512], F32, tag="pn1")
            pn = [pn0, pn1]
            for bg in range(NBIG):
                nc.vector.memset(pn[bg][:], 0.0)

            # ---- PE-share: f-layout pair-diff + abs + PE k-reduce ----
            absd = [cst.tile([128, PB], BF16, tag=f"absd{t}",
                             name=f"absd{t}")
                    for t in range(NCI)]

            def emit_pe_tile(t):
                for lo in range(0, PB, 1024):
                    w = min(1024, PB - lo)
                    pdt = pdiff.tile([128, F], F32, tag="pd")
                    for h in range(0, w, 512):
                        hw = min(512, w - h)
                        nc.tensor.matmul(pdt[:, h:h + hw],
                                         m8[:, 128 * t:128 * (t + 1)],
                                         psel_sb[:, lo + h:lo + h + hw],
                                         start=True, stop=True)
                    nc.scalar.activation(absd[t][:, lo:lo + w], pdt[:, 0:w],
                                         AF.Abs)
                for i in range(I0):
                    bg, sub, ih = i // 8, (i % 8) // 2, i % 2
                    bs = i * 127 - (i * (i - 1)) // 2
                    w = 127 - i
                    out_ap = pn[bg][64 * ih:64 * ih + 64,
                                    128 * sub + i + 1:128 * (sub + 1)]
                    nc.tensor.matmul(out_ap, s64_sb[:],
                                     absd[t][:, bs:bs + w],
                                     start=(t == 0), stop=(t == NCI - 1),
                                     tile_position=(0, 64 * ih),
                                     skip_group_check=True)

            rs_all = cst.tile([128, 4 * NBIG], F32, tag="rs")

            def emit_pe_finish(po_flags):
                for bg in range(NBIG):
                    e = ep.tile([128, 512], BF16, tag="ebig")
                    nc.scalar.activation(e[:], pn[bg][:], AF.Exp, scale=-1.0)
                    for sub in range(4):
                        st, sp = po_flags.pop(0)
                        nc.tensor.matmul(po[:], s64_sb[:],
                                         e[:, 128 * sub:128 * (sub + 1)],
                                         start=st, stop=sp)
                    nc.vector.tensor_reduce(
                        rs_all[:, 4 * bg:4 * (bg + 1)],
                        e[:].rearrange("p (s j) -> p s j", s=4),
                        op=A.add, axis=mybir.AxisListType.X)

            # ---- pairs-share groups ----
            g_chunk0 = [sum(GROUPS[:g]) for g in range(len(GROUPS))]

            def emit_group_pd_abs(g):
                gs = GROUPS[g]
                ab = abp.tile([128, N_ACT * F], BF16, tag="ab")
                nrm = fold.tile([128, gs * OC], BF16, tag=f"nrm{gs}")
                # emit ACT(cc<N_ACT) and DVE(cc>=N_ACT) chunks interleaved
                # so both consumers drain pd tiles from the start
                order = []
                na, nd = 0, N_ACT
                for k in range(gs):
                    if (k % 3 == 2 and nd < gs) or na >= N_ACT:
                        order.append(nd)
                        nd += 1
                    else:
                        order.append(na)
                        na += 1
                for cc in order:
                    c = g_chunk0[g] + cc
                    pd = pdiff.tile([128, F], F32, tag="pd")
                    for h in range(2):
                        nc.tensor.matmul(
                            pd[:, 512 * h:512 * (h + 1)],
                            psel_sb[:, PB + 128 * c:PB + 128 * (c + 1)],
                            m8[:, 512 * h:512 * (h + 1)],
                            start=True, stop=True)
                    if cc < N_ACT:
                        nc.scalar.activation(ab[:, F * cc:F * (cc + 1)],
                                             pd[:], AF.Abs)
                    else:
                        with nc.allow_low_precision(reason="norm in bf16"):
                            nc.vector.tensor_reduce(
                                nrm[:, OC * cc:OC * (cc + 1)],
                                pd[:].rearrange("p (k o) -> p o k", k=K),
                                op=A.add, axis=mybir.AxisListType.X,
                                apply_absolute_value=True)
                return ab, nrm

            def emit_group_reduce(g, ab, nrm, po_flags):
                gs = GROUPS[g]
                v = ab[:].rearrange("p (c k o) -> p c k o", c=N_ACT, k=K)
                n1 = fold.tile([128, N_ACT * 8 * OC], BF16, tag="n1")
                v1 = n1[:].rearrange("p (c k o) -> p c k o", c=N_ACT, k=8)
                n2 = fold.tile([128, N_ACT * 4 * OC], BF16, tag="n2")
                v2 = n2[:].rearrange("p (c k o) -> p c k o", c=N_ACT, k=4)
                n3 = fold.tile([128, N_ACT * 2 * OC], BF16, tag="n3")
                v3 = n3[:].rearrange("p (c k o) -> p c k o", c=N_ACT, k=2)
                vn = nrm[:, 0:N_ACT * OC].rearrange(
                    "p (c k o) -> p c k o", c=N_ACT, k=1)
                with nc.allow_low_precision(reason="norm folds in bf16"):
                    nc.vector.tensor_tensor(v1, v[:, :, 0:8], v[:, :, 8:16],
                                            op=A.add)
                    nc.vector.tensor_tensor(v2, v1[:, :, 0:4], v1[:, :, 4:8],
                                            op=A.add)
                    nc.vector.tensor_tensor(v3, v2[:, :, 0:2], v2[:, :, 2:4],
                                            op=A.add)
                    nc.vector.tensor_tensor(vn, v3[:, :, 0:1], v3[:, :, 1:2],
                                            op=A.add)
                e = ep.tile([128, gs * OC], F8, tag=f"e{gs}")
                nc.scalar.activation(e[:], nrm[:], AF.Exp, scale=-1.0)
                for cc in range(gs):
                    c = g_chunk0[g] + cc
                    st, sp = po_flags.pop(0)
                    nc.tensor.matmul(
                        po[:], e[:, OC * cc:OC * (cc + 1)],
                        zt_sb[:, B * c:B * (c + 1)],
                        start=st, stop=sp)

            # po accumulation flags, in emission order: g0 jsums (13),
            # g1 (12), g2 (12), PE colsums (4*NBIG), g3 jsums (12)
            n_po = NCHUNK + 4 * NBIG
            po_flags = [(k == 0, k == n_po - 1) for k in range(n_po)]

            # schedule: interleave PE-share tiles with pairs groups
            ab0 = emit_group_pd_abs(0)
            emit_pe_tile(0)
            emit_pe_tile(1)
            ab1 = emit_group_pd_abs(1)
            emit_group_reduce(0, *ab0, po_flags)
            emit_pe_tile(2)
            emit_pe_tile(3)
            ab2 = emit_group_pd_abs(2)
            emit_group_reduce(1, *ab1, po_flags)
            emit_pe_tile(4)
            emit_pe_tile(5)
            ab3 = emit_group_pd_abs(3)
            emit_group_reduce(2, *ab2, po_flags)
            emit_pe_tile(6)
            emit_pe_tile(7)
            emit_pe_finish(po_flags)
            emit_group_reduce(3, *ab3, po_flags)
            assert not po_flags

            po_sb = cst.tile([OC, B], F32, tag="posb")
            nc.vector.tensor_copy(po_sb[:], po[:])
            nc.sync.dma_start(po_d[:], po_sb[:])
            nc.sync.dma_start(rs_d[:], rs_all[:])

    _split_excess_waits(nc)
    return nc


def _host_consts():
    ii, jj = np.triu_indices(B, k=1)      # i-major pair order
    psel = np.zeros((B, PSEL_COLS), np.float32)
    p = np.arange(NPAIR)
    psel[ii, p] = 1.0
    psel[jj, p] = -1.0
    zt = np.zeros((128, NCHUNK * B), np.float32)
    ps = p[PB:] - PB
    c, r = ps // 128, ps % 128
    zt[r, c * B + ii[PB:]] = 1.0
    zt[r, c * B + jj[PB:]] = 1.0
    s64 = np.zeros((128, OC), np.float32)
    s64[np.arange(128), np.arange(128) % OC] = 1.0
    return (psel.astype(NP_F8), zt.astype(NP_F8),
            s64.astype(ml_dtypes.bfloat16))


_CACHE = {}


def _get_cached():
    if "nc" not in _CACHE:
        _CACHE["nc"] = _build_program()
        _CACHE["consts"] = _host_consts()
    return _CACHE


def kernel(x: np.ndarray, T: np.ndarray, _trace=False, _tmpdir=None) -> np.ndarray:
    x = np.asarray(x, dtype=np.float32)
    T = np.asarray(T, dtype=np.float32)
    c = _get_cached()
    nc = c["nc"]
    psel, zt, s64 = c["consts"]

    xt8 = np.ascontiguousarray(x.T).astype(NP_F8)
    in_maps = []
    for cr in range(NCORES):
        tc8 = np.ascontiguousarray(
            T[:, OC * cr:OC * (cr + 1), :].transpose(0, 2, 1).reshape(IN, F)
        ).astype(NP_F8)
        in_maps.append({"xt": xt8, "tc": tc8, "psel": psel, "zt": zt,
                        "s64": s64})

    kw = {}
    if _trace:
        kw = dict(trace=True, tmpdir=_tmpdir)
    res = run_bass_kernel_spmd(nc, in_maps, list(range(NCORES)), **kw)

    jcol = np.arange(B, dtype=np.float32)
    junk_col = np.maximum(0.0, I0 - jcol)[None, :]      # [1, 128]
    i_idx = np.arange(I0)
    rs_rows = 64 * (i_idx % 2)                          # + o
    rs_cols = 4 * (i_idx // 8) + (i_idx % 8) // 2
    o_b = np.empty((B, OUT), np.float32)
    for cr in range(NCORES):
        r = res.results[cr]
        ob_c = (r["po"] - junk_col).T.copy()            # [j, o_local]
        rs = r["rs"]                                    # [128, 4*NBIG]
        for i in range(I0):
            ob_c[i, :] += (rs[rs_rows[i]:rs_rows[i] + OC, rs_cols[i]]
                           - (i + 1))
        o_b[:, OC * cr:OC * (cr + 1)] = ob_c
    out = np.concatenate([x, o_b], axis=1)
    if _trace:
        return out, res
    return out


# revision 10
# speedup vs baseline: 1.1390x; 1.0203x over previous
"""MinibatchDiscrimination kernel for 8 Trainium2 NeuronCores (v3 hybrid).

reference:
    m = einsum('bi,iok->bok', x, T)          # B=128, IN=1024, OUT=512, K=16
    norm[i,j,o] = sum_k |m[j,o,k] - m[i,o,k]|
    o_b = sum_i exp(-norm) - 1               # [B, OUT]
    out = concat([x, o_b], axis=1)           # [128, 1536]

Sharding: each core owns OUT/8 = 64 output features (zero communication).
f = k*64 + o_local (k-major).

Hybrid split of the 8128 strict-upper pairs:
- PE-share: pairs with i < I0 (p < PB) in f-layout. Pair-diff per f-tile
  t: absd_t [128f, PB] (ACT abs). k-reduce ON THE PE: per (i, t) one
  matmul with the fixed selector s64[p, o]=1{p%64==o}, accumulating all 8
  tiles into packed PSUM pn[big] [128=2i*64o, 512=4sub*128j]. exp on ACT,
  column sums via s64 again into po, row sums via DVE tensor_reduce.
  Host subtracts deterministic junk (memset cells j<=i read exp(0)=1).
- pairs-share: remaining pairs in pairs-on-partitions chunks of 128.
  Per chunk: pair-diff matmul -> pd [128pair, 1024f] PSUM; first N_ACT
  chunks of each group: ACT abs -> DVE fold tree; rest: one fused DVE
  tensor_reduce(|.|, sum k). exp(-norm) on ACT -> e fp8; j-sum matmul
  po += e_c^T @ zt_c (zt has 1s at both i and j; pad pairs are zero).
Host: o_b[j, o] = po[o, j] - junk + rowsum scatter; out = concat(x, o_b).
"""

import numpy as np
import ml_dtypes

import concourse.bass as bass
import concourse.tile as tile
from concourse import mybir
from concourse.bass_utils import run_bass_kernel_spmd

BF16 = mybir.dt.bfloat16
F32 = mybir.dt.float32
F8 = mybir.dt.float8e4
NP_F8 = ml_dtypes.float8_e4m3
A = mybir.AluOpType
AF = mybir.ActivationFunctionType

B = 128
IN = 1024
OUT = 512
K = 16
NCORES = 8
OC = OUT // NCORES       # 64
F = OC * K               # 1024 (f = k*64 + o)
NCI = IN // 128          # 8
NPAIR = (B * (B - 1)) // 2   # 8128

I0 = 16                  # PE-share: pairs with i < I0
PB = I0 * 127 - (I0 * (I0 - 1)) // 2   # 1912 pairs
NBIG = (I0 + 7) // 8     # pn tiles (8 i's each)

NP_PAIRS = NPAIR - PB    # 6216 pairs-share pairs
NCHUNK = (NP_PAIRS + 127) // 128       # 49 chunks
PSEL_COLS = PB + NCHUNK * 128          # 8184
GROUPS = (13, 12, 12, 12)              # chunks per fold group
N_ACT = 6                              # ACT-abs chunks per group


def _split_excess_waits(nc, max_waits=1):
    """This walrus build rejects instructions carrying more than one sem
    wait; hoist extras onto preceding NoOps on the same engine."""
    for fn in nc.m.functions:
        for blk in fn.blocks:
            new_insts = []
            for inst in blk.instructions:
                si = inst.sync_info
                if si and si.on_wait and len(si.on_wait) > max_waits:
                    waits = list(si.on_wait)
                    extra, keep = waits[:-max_waits], waits[-max_waits:]
                    k = 0
                    while extra:
                        chunk, extra = extra[:max_waits], extra[max_waits:]
                        nop = mybir.InstNoOp(
                            name=f"{inst.name}-ws{k}", engine=inst.engine,
                            ins=[], outs=[],
                            sync_info=mybir.SyncInfo(on_wait=chunk, on_update=[]))
                        nc.register_instruction(nop)
                        new_insts.append(nop)
                        k += 1
                    inst.sync_info = mybir.SyncInfo(
                        on_wait=keep, on_update=list(si.on_update))
                new_insts.append(inst)
            blk.instructions[:] = new_insts


def _build_program():
    nc = bass.Bass()
    xT_d = nc.dram_tensor("xt", [IN, B], F8, kind="ExternalInput")
    tc_d = nc.dram_tensor("tc", [IN, F], F8, kind="ExternalInput")
    psel_d = nc.dram_tensor("psel", [B, PSEL_COLS], F8, kind="ExternalInput")
    zt_d = nc.dram_tensor("zt", [128, NCHUNK * B], F8, kind="ExternalInput")
    s64_d = nc.dram_tensor("s64", [128, OC], BF16, kind="ExternalInput")
    po_d = nc.dram_tensor("po", [OC, B], F32, kind="ExternalOutput")
    rs_d = nc.dram_tensor("rs", [128, 4 * NBIG], F32, kind="ExternalOutput")

    with tile.TileContext(nc) as tc:
        with (
            tc.tile_pool(name="cst", bufs=1) as cst,
            tc.tile_pool(name="ab", bufs=2) as abp,
            tc.tile_pool(name="fold", bufs=2) as fold,
            tc.tile_pool(name="ep", bufs=2) as ep,
            tc.tile_pool(name="pdiff", bufs=2, space="PSUM") as pdiff,
            tc.tile_pool(name="pn", bufs=1, space="PSUM") as pnp,
            tc.tile_pool(name="pob", bufs=1, space="PSUM") as pob,
        ):
            # Few large DMAs: the ~600ns/descriptor fixed latency dominates
            # small transfers. xt/tc land as single tiles via 3D APs.
            s64_sb = cst.tile([128, OC], BF16, tag="s64")
            nc.sync.dma_start(s64_sb[:], s64_d[:, :])
            xt_sb = cst.tile([128, NCI * B], F8, tag="xt")
            nc.sync.dma_start(
                xt_sb[:].rearrange("p (c b) -> p c b", c=NCI),
                xT_d[:].rearrange("(c p) b -> p c b", p=128))
            tc_sb = cst.tile([128, NCI * F], F8, tag="tc")
            tcv = tc_sb[:].rearrange("p (c f) -> p c f", c=NCI)
            tdv = tc_d[:].rearrange("(c p) f -> p c f", p=128)
            nc.sync.dma_start(tcv[:, 0:4], tdv[:, 0:4])
            nc.sync.dma_start(tcv[:, 4:8], tdv[:, 4:8])
            psel_sb = cst.tile([128, PSEL_COLS], F8, tag="psel")
            for lo in range(0, PSEL_COLS, 2048):
                w = min(2048, PSEL_COLS - lo)
                nc.sync.dma_start(psel_sb[:, lo:lo + w],
                                  psel_d[:, lo:lo + w])
            zt_sb = cst.tile([128, NCHUNK * B], F8, tag="zt")
            for lo in range(0, NCHUNK * B, 2048):
                w = min(2048, NCHUNK * B - lo)
                nc.sync.dma_start(zt_sb[:, lo:lo + w], zt_d[:, lo:lo + w])

            # ---- GEMM: m8[b, f] = x8 @ T8 (psum tile from the pd pool).
            # half0 chain + cast first so pair-diff h=0 matmuls (and the
            # first 4 PE-share tiles) start while half1 still runs.
            m8 = cst.tile([128, F], F8, tag="m8")
            ps = pdiff.tile([128, F], F32, tag="pd")
            for half in range(2):
                for ci in range(NCI):
                    nc.tensor.matmul(
                        ps[:, 512 * half:512 * (half + 1)],
                        xt_sb[:, B * ci:B * (ci + 1)],
                        tc_sb[:, F * ci + 512 * half:F * ci + 512 * (half + 1)],
                        start=(ci == 0), stop=(ci == NCI - 1),
                        skip_group_check=True)
                nc.scalar.activation(m8[:, 512 * half:512 * (half + 1)],
                                     ps[:, 512 * half:512 * (half + 1)],
                                     AF.Copy, scale=1.0)

            po = pob.tile([OC, B], F32, tag="po")
            pn0 = pnp.tile([128, 512], F32, tag="pn0")
            pn1 = pnp.tile([128, 512], F32, tag="pn1")
            pn = [pn0, pn1]
            for bg in range(NBIG):
                nc.vector.memset(pn[bg][:], 0.0)

            # ---- PE-share: f-layout pair-diff + abs + PE k-reduce ----
            absd = [cst.tile([128, PB], BF16, tag=f"absd{t}",
                             name=f"absd{t}")
                    for t in range(NCI)]

            def pe_tile_steps(t):
                """PE-share work for f-tile t as weavable closures:
                2x (pair-diff + abs), then 4x (4 k-reduce matmuls)."""
                def pd_abs(lo):
                    w = min(1024, PB - lo)
                    pdt = pdiff.tile([128, F], F32, tag="pd")
                    for h in range(0, w, 512):
                        hw = min(512, w - h)
                        nc.tensor.matmul(pdt[:, h:h + hw],
                                         m8[:, 128 * t:128 * (t + 1)],
                                         psel_sb[:, lo + h:lo + h + hw],
                                         start=True, stop=True)
                    nc.scalar.activation(absd[t][:, lo:lo + w], pdt[:, 0:w],
                                         AF.Abs)

                def kred(i4):
                    for i in range(i4, min(i4 + 4, I0)):
                        bg, sub, ih = i // 8, (i % 8) // 2, i % 2
                        bs = i * 127 - (i * (i - 1)) // 2
                        w = 127 - i
                        out_ap = pn[bg][64 * ih:64 * ih + 64,
                                        128 * sub + i + 1:128 * (sub + 1)]
                        # start=False always: pn is memset-zeroed, and a
                        # start=True here could stomp sibling i-chains
                        # sharing the bank's column range
                        nc.tensor.matmul(out_ap, s64_sb[:],
                                         absd[t][:, bs:bs + w],
                                         start=False, stop=(t == NCI - 1),
                                         tile_position=(0, 64 * ih),
                                         skip_group_check=True)

                return ([lambda lo=lo: pd_abs(lo)
                         for lo in range(0, PB, 1024)] +
                        [lambda i4=i4: kred(i4) for i4 in range(0, I0, 4)])

            rs_all = cst.tile([128, 4 * NBIG], F32, tag="rs")

            def emit_pe_finish(po_flags):
                for bg in range(NBIG):
                    e = ep.tile([128, 512], BF16, tag="ebig")
                    nc.scalar.activation(e[:], pn[bg][:], AF.Exp, scale=-1.0)
                    for sub in range(4):
                        st, sp = po_flags.pop(0)
                        nc.tensor.matmul(po[:], s64_sb[:],
                                         e[:, 128 * sub:128 * (sub + 1)],
                                         start=st, stop=sp)
                        # plain per-sub row sums (grouped reduce into a
                        # sliced output landed only the last group)
                        nc.vector.tensor_reduce(
                            rs_all[:, 4 * bg + sub:4 * bg + sub + 1],
                            e[:, 128 * sub:128 * (sub + 1)],
                            op=A.add, axis=mybir.AxisListType.X)

            # ---- pairs-share groups ----
            g_chunk0 = [sum(GROUPS[:g]) for g in range(len(GROUPS))]

            def emit_group_pd_abs(g, weave):
                gs = GROUPS[g]
                ab = abp.tile([128, N_ACT * F], BF16, tag="ab")
                nrm = fold.tile([128, gs * OC], BF16, tag=f"nrm{gs}")
                # emit ACT(cc<N_ACT) and DVE(cc>=N_ACT) chunks interleaved
                # so both consumers drain pd tiles from the start
                order = []
                na, nd = 0, N_ACT
                for k in range(gs):
                    if (k % 2 == 1 and nd < gs) or na >= N_ACT:
                        order.append(nd)
                        nd += 1
                    else:
                        order.append(na)
                        na += 1
                for cc in order:
                    if weave:
                        weave.pop(0)()
                    c = g_chunk0[g] + cc
                    pd = pdiff.tile([128, F], F32, tag="pd")
                    for h in range(2):
                        nc.tensor.matmul(
                            pd[:, 512 * h:512 * (h + 1)],
                            psel_sb[:, PB + 128 * c:PB + 128 * (c + 1)],
                            m8[:, 512 * h:512 * (h + 1)],
                            start=True, stop=True)
                    if cc < N_ACT:
                        nc.scalar.activation(ab[:, F * cc:F * (cc + 1)],
                                             pd[:], AF.Abs)
                    else:
                        with nc.allow_low_precision(reason="norm in bf16"):
                            nc.vector.tensor_reduce(
                                nrm[:, OC * cc:OC * (cc + 1)],
                                pd[:].rearrange("p (k o) -> p o k", k=K),
                                op=A.add, axis=mybir.AxisListType.X,
                                apply_absolute_value=True)
                return ab, nrm

            def emit_group_jsum(g, e, po_flags):
                gs = GROUPS[g]
                for cc in range(gs):
                    c = g_chunk0[g] + cc
                    st, sp = po_flags.pop(0)
                    nc.tensor.matmul(
                        po[:], e[:, OC * cc:OC * (cc + 1)],
                        zt_sb[:, B * c:B * (c + 1)],
                        start=st, stop=sp)

            def emit_group_reduce(g, ab, nrm, po_flags):
                gs = GROUPS[g]
                v = ab[:].rearrange("p (c k o) -> p c k o", c=N_ACT, k=K)
                n1 = fold.tile([128, N_ACT * 8 * OC], BF16, tag="n1")
                v1 = n1[:].rearrange("p (c k o) -> p c k o", c=N_ACT, k=8)
                n2 = fold.tile([128, N_ACT * 4 * OC], BF16, tag="n2")
                v2 = n2[:].rearrange("p (c k o) -> p c k o", c=N_ACT, k=4)
                n3 = fold.tile([128, N_ACT * 2 * OC], BF16, tag="n3")
                v3 = n3[:].rearrange("p (c k o) -> p c k o", c=N_ACT, k=2)
                vn = nrm[:, 0:N_ACT * OC].rearrange(
                    "p (c k o) -> p c k o", c=N_ACT, k=1)
                with nc.allow_low_precision(reason="norm folds in bf16"):
                    nc.vector.tensor_tensor(v1, v[:, :, 0:8], v[:, :, 8:16],
                                            op=A.add)
                    nc.vector.tensor_tensor(v2, v1[:, :, 0:4], v1[:, :, 4:8],
                                            op=A.add)
                    nc.vector.tensor_tensor(v3, v2[:, :, 0:2], v2[:, :, 2:4],
                                            op=A.add)
                    nc.vector.tensor_tensor(vn, v3[:, :, 0:1], v3[:, :, 1:2],
                                            op=A.add)
                e = ep.tile([128, gs * OC], F8, tag=f"e{gs}")
                nc.scalar.activation(e[:], nrm[:], AF.Exp, scale=-1.0)
                if po_flags is None:
                    return e
                emit_group_jsum(g, e, po_flags)

            # po accumulation flags, in emission order: g0 jsums (13),
            # g1 (12), g2 (12), PE colsums (4*NBIG), g3 jsums (12)
            n_po = NCHUNK + 4 * NBIG
            po_flags = [(k == 0, k == n_po - 1) for k in range(n_po)]

            # schedule: weave PE-share steps (8 tiles x 6) between the 49
            # pairs-chunk emissions so the PE never monopolizes long
            # stretches and ACT/DVE stay fed
            steps = []
            for t in range(NCI):
                steps += pe_tile_steps(t)
            ab0 = emit_group_pd_abs(0, steps)
            ab1 = emit_group_pd_abs(1, steps)
            emit_group_reduce(0, *ab0, po_flags)
            ab2 = emit_group_pd_abs(2, steps)
            emit_group_reduce(1, *ab1, po_flags)
            ab3 = emit_group_pd_abs(3, steps)
            emit_group_reduce(2, *ab2, po_flags)
            for s in steps:
                s()
            emit_pe_finish(po_flags)
            emit_group_reduce(3, *ab3, po_flags)
            assert not po_flags

            po_sb = cst.tile([OC, B], F32, tag="posb")
            nc.vector.tensor_copy(po_sb[:], po[:])
            nc.sync.dma_start(po_d[:], po_sb[:])
            nc.sync.dma_start(rs_d[:], rs_all[:])

    _split_excess_waits(nc)
    return nc


def _host_consts():
    ii, jj = np.triu_indices(B, k=1)      # i-major pair order
    psel = np.zeros((B, PSEL_COLS), np.float32)
    p = np.arange(NPAIR)
    psel[ii, p] = 1.0
    psel[jj, p] = -1.0
    zt = np.zeros((128, NCHUNK * B), np.float32)
    ps = p[PB:] - PB
    c, r = ps // 128, ps % 128
    zt[r, c * B + ii[PB:]] = 1.0
    zt[r, c * B + jj[PB:]] = 1.0
    s64 = np.zeros((128, OC), np.float32)
    s64[np.arange(128), np.arange(128) % OC] = 1.0
    return (psel.astype(NP_F8), zt.astype(NP_F8),
            s64.astype(ml_dtypes.bfloat16))


_CACHE = {}


def _get_cached():
    if "nc" not in _CACHE:
        _CACHE["nc"] = _build_program()
        _CACHE["consts"] = _host_consts()
    return _CACHE


def kernel(x: np.ndarray, T: np.ndarray, _trace=False, _tmpdir=None) -> np.ndarray:
    x = np.asarray(x, dtype=np.float32)
    T = np.asarray(T, dtype=np.float32)
    c = _get_cached()
    nc = c["nc"]
    psel, zt, s64 = c["consts"]

    xt8 = np.ascontiguousarray(x.T).astype(NP_F8)
    in_maps = []
    for cr in range(NCORES):
        tc8 = np.ascontiguousarray(
            T[:, OC * cr:OC * (cr + 1), :].transpose(0, 2, 1).reshape(IN, F)
        ).astype(NP_F8)
        in_maps.append({"xt": xt8, "tc": tc8, "psel": psel, "zt": zt,
                        "s64": s64})

    kw = {}
    if _trace:
        kw = dict(trace=True, tmpdir=_tmpdir)
    res = run_bass_kernel_spmd(nc, in_maps, list(range(NCORES)), **kw)

    jcol = np.arange(B, dtype=np.float32)
    junk_col = np.maximum(0.0, I0 - jcol)[None, :]      # [1, 128]
    i_idx = np.arange(I0)
    rs_rows = 64 * (i_idx % 2)                          # + o
    rs_cols = 4 * (i_idx // 8) + (i_idx % 8) // 2
    o_b = np.empty((B, OUT), np.float32)
    for cr in range(NCORES):
        r = res.results[cr]
        ob_c = (r["po"] - junk_col).T.copy()            # [j, o_local]
        rs = r["rs"]                                    # [128, 4*NBIG]
        for i in range(I0):
            ob_c[i, :] += (rs[rs_rows[i]:rs_rows[i] + OC, rs_cols[i]]
                           - (i + 1))
        o_b[:, OC * cr:OC * (cr + 1)] = ob_c
    out = np.concatenate([x, o_b], axis=1)
    if _trace:
        return out, res
    return out
